# revision 1
# baseline (speedup 1.0000x reference)
"""Trainium2 Bass kernel for BiDAF-style bidirectional attention.

Reference computation (per batch element n; M=1 folded away):
    s[i,j]  = h[i].w_h + u[j].w_u + (h[i]*u[j]).w_hu + b      [JX, JQ]
    a_u     = softmax_j(s);     u_a[i] = sum_j a_u[i,j] u[j]   (c2q)
    a_h     = softmax_i(max_j s);  h_a = sum_i a_h[i] h[i]     (q2c)
    out     = concat(h, u_a, h*u_a, h*h_a)                     [JX, 4D]

Sharding: data-parallel over batch N=8, one NeuronCore per batch element.
alpha_b drops out of the output entirely (it shifts s by a constant, and both
softmaxes are shift-invariant), so it is accepted but unused.

Per-core dataflow (i = context position, j = query position, d = feature):
  - h arrives [JX, D] row-major; matmuls contracting over d need h^T, built
    with 32 PE transposes (4 per PSUM bank, one batched ScalarE evict each).
  - scores are computed TRANSPOSED: s0T[j, i] = sum_d (u*w_hu)[j,d] h[i,d]
    via lhsT=uwT chunks, rhs=hT chunks, accumulating 4 d-chunks in PSUM.
    h.w_h is folded in with one extra K=1 matmul (ones_row outer hwh_row);
    u.w_u is folded in as the per-partition bias of the ScalarE Exp that
    evicts PSUM->SBUF: ET = exp(s0T + uwu[j]).  exp(hwh[i]) scales whole
    rows i of ET, which cancels in the j-softmax, and keeps max_j exact.
  - c2q: PE re-transposes ET (4 tiles per PSUM bank); one 3D DVE reduce per
    block gives row maxes/sums; u_a = (ET_tile^T @ u) scaled by 1/rowsum on
    DVE into a staging buffer shared with o3 = h*u_a (one DMA per tile).
  - q2c: weights w_i = max_j exp(...) = exp(max_j s - b); h_a via per-block
    M=1 f32r matmuls (block 0's overlap block 1's score work); broadcast
    back with a K=1 matmul; o4 muls split between DVE and GpSimd.
Engine balance: PE matmuls/transposes, ScalarE exp + PSUM evictions, DVE
reduces + normalize + output muls, GpSimd f32r copies + h passthrough DMAs.
A plain-f32 PE warmup burst (no cross-engine deps) lifts the HAM clock gate
to 2.4 GHz while the h DMAs are still in flight.
"""

import numpy as np

N_B, M_B, JX, JQ, D = 8, 1, 1024, 128, 512
P = 128
NT = JX // P   # 8 i-tiles
KC = D // P    # 4 d-chunks
IB = 512       # i-block width for score matmuls
NB = JX // IB  # 2 blocks
TPB = NT // NB  # tiles per block

_CACHE = {}


def _build_program():
    from contextlib import ExitStack

    import concourse.bass as bass
    import concourse.tile as tile
    from concourse import bacc, mybir
    from concourse.masks import make_identity
    from concourse.tile_rust import add_dep_helper

    f32 = mybir.dt.float32
    f32r = mybir.dt.float32r
    EXP = mybir.ActivationFunctionType.Exp
    AX = mybir.AxisListType.X
    ds = bass.ds

    nc = bacc.Bacc("TRN2", target_bir_lowering=False, debug=False, num_devices=8)
    h_d = nc.dram_tensor("h", [JX, D], f32, kind="ExternalInput").ap()
    u_d = nc.dram_tensor("u", [JQ, D], f32, kind="ExternalInput").ap()
    aw_d = nc.dram_tensor("alpha_w", [3 * D], f32, kind="ExternalInput").ap()
    out_d = nc.dram_tensor("out", [JX, 4 * D], f32, kind="ExternalOutput").ap()

    with tile.TileContext(nc) as tc, ExitStack() as ctx:
        consts = ctx.enter_context(tc.tile_pool(name="consts", bufs=1))
        stage = ctx.enter_context(tc.tile_pool(name="stage", bufs=6))
        # PSUM budget (8 banks): tp=2, s0=2, ua=2, acc=1, hap=1
        ps = ctx.enter_context(tc.tile_pool(name="ps", bufs=2, space="PSUM"))

        # ---- PE warmup: f32r N=512 matmuls depending only on DVE ops,
        # emitted first so the HAM clock-gate opens (1.2 -> 2.4 GHz) while
        # the h DMAs stream in (~630 ns each cold, ~6.3 us of PE busy).
        warm_f = consts.tile([P, D], f32)
        nc.vector.memset(warm_f[:], 0.25)
        warm = consts.tile([P, D], f32r)
        nc.vector.tensor_copy(warm[:], warm_f[:])
        wp = ps.tile([P, D], f32, tag="acc", bufs=1)
        for w in range(16):
            nc.tensor.matmul(
                wp[:], warm[:, ds(0, P)], warm[:], start=True, stop=True,
            )

        # ---- constants / prep ----
        ident = consts.tile([P, P], f32)
        make_identity(nc, ident[:])
        ident_r = consts.tile([P, P], f32r)
        nc.vector.tensor_copy(ident_r[:], ident[:])
        ones_row = consts.tile([1, P], f32)
        nc.vector.memset(ones_row[:], 1.0)
        ones_row_r = consts.tile([1, P], f32r)
        nc.scalar.copy(ones_row_r[:], ones_row[:])
        ones_col = consts.tile([P, 1], f32)
        nc.vector.memset(ones_col[:], 1.0)

        u_sb = consts.tile([JQ, D], f32)
        nc.sync.dma_start(u_sb[:], u_d[:])
        u_r = consts.tile([JQ, D], f32r)
        nc.scalar.copy(u_r[:], u_sb[:])
        w_cols = consts.tile([P, 12], f32)  # alpha_w partition-major: d = c*128+p
        nc.sync.dma_start(w_cols[:], aw_d.rearrange("(c p) -> p c", p=P))
        w_cols_r = consts.tile([P, 12], f32r)
        nc.vector.tensor_copy(w_cols_r[:], w_cols[:])
        wb = consts.tile([P, 2 * D], f32)  # [w_u | w_hu] broadcast across partitions
        nc.sync.dma_start(
            wb[:], aw_d[ds(D, 2 * D)].rearrange("(o d) -> o d", o=1).to_broadcast((P, 2 * D))
        )
        wu_b = wb[:, ds(0, D)]
        whu_b = wb[:, ds(D, D)]

        # uw[j,d] = u[j,d]*w_hu[d];  uwu[j] = sum_d u[j,d]*w_u[d]
        uw = consts.tile([JQ, D], f32)
        nc.vector.tensor_mul(uw[:], u_sb[:], whu_b)
        uwtmp = consts.tile([JQ, D], f32)
        uwu = consts.tile([JQ, 1], f32)
        nc.vector.scalar_tensor_tensor(
            uwtmp[:], u_sb[:], 1.0, wu_b,
            op0=mybir.AluOpType.mult, op1=mybir.AluOpType.mult, accum_out=uwu[:],
        )

        # uwT[d_chunk][j]: 4 transposes into one PSUM bank, one batched evict
        uwT = consts.tile([P, KC * JQ], f32r)
        pt = ps.tile([P, KC * P], f32, tag="tp")
        for k in range(KC):
            nc.tensor.transpose(pt[:, ds(k * P, P)], uw[:, ds(k * P, P)], ident[:])
        nc.scalar.copy(uwT[:], pt[:])

        # ---- load h; passthrough out1; build hT ----
        h_all = consts.tile([P, NT * D], f32)    # tile t: h[t*128+p, d]
        h_r = consts.tile([P, NT * D], f32r)
        hT_all = consts.tile([P, KC * JX], f32r)  # chunk k: hT[k*128+p, i]
        hT3 = hT_all[:].rearrange("p (k x) -> p k x", k=KC)
        hout_late = []
        for t in range(NT):
            nc.sync.dma_start(h_all[:, ds(t * D, D)], h_d[ds(t * P, P), :])
            # out1 = h passthrough (GpSimd DMA queue; Sync stays free).  The
            # later tiles are gated on block-0's exp (below) so ~1 MB of
            # passthrough lands in the mid-kernel DMA lull instead of
            # competing with the h loads.
            ho = nc.gpsimd.dma_start(out_d[ds(t * P, P), ds(0, D)], h_all[:, ds(t * D, D)])
            if t >= NT // 2:
                hout_late.append(ho)
        def transpose_tiles(ts_range):
            for t in ts_range:
                pt = ps.tile([P, KC * P], f32, tag="tp")
                for k in range(KC):
                    nc.tensor.transpose(
                        pt[:, ds(k * P, P)], h_all[:, ds(t * D + k * P, P)], ident[:]
                    )
                ev = nc.scalar.copy if t % 2 == 0 else nc.vector.tensor_copy
                ev(hT3[:, :, ds(t * P, P)], pt[:].rearrange("p (k x) -> p k x", k=KC))

        transpose_tiles(range(0, NT))


        # ---- scores (transposed), exp, c2q, per-block q2c accumulation ----
        hwh_row = consts.tile([1, JX], f32r)      # h.w_h as a row over i
        ET = consts.tile([JQ, JX], f32r)          # exp(s0T + uwu[j]) (row-scaled)
        m_exp = consts.tile([P, NT], f32)         # per i-tile: max_j ET
        m_exp_r = consts.tile([P, NT], f32r)
        z_rec = consts.tile([P, NT], f32)         # per i-tile: 1/sum_j ET
        hap = ps.tile([1, D], f32, tag="hap", bufs=1)

        for b in range(NB):
            blk = ds(b * IB, IB)
            for q in range(TPB):
                t = b * TPB + q
                nc.scalar.copy(h_r[:, ds(t * D, D)], h_all[:, ds(t * D, D)])
            # hwh chunk: [1, IB] row accumulated over d-chunks
            hp = ps.tile([1, IB], f32, tag="acc", bufs=1)
            for k in range(KC):
                nc.tensor.matmul(
                    hp[:], w_cols_r[:, ds(k, 1)], hT_all[:, ds(k * JX + b * IB, IB)],
                    start=(k == 0), stop=(k == KC - 1),
                )
            nc.scalar.copy(hwh_row[:, blk], hp[:])

            sp = ps.tile([JQ, IB], f32, tag="s0")
            for k in range(KC):
                nc.tensor.matmul(
                    sp[:], uwT[:, ds(k * JQ, JQ)], hT_all[:, ds(k * JX + b * IB, IB)],
                    start=(k == 0), stop=False,
                )
            nc.tensor.matmul(
                sp[:], ones_row_r[:], hwh_row[:, blk], start=False, stop=True
            )
            # ET = exp(s0T + uwu[j]); uwu is the per-partition (j) ACT bias
            exp_inst = nc.scalar.activation(ET[:, blk], sp[:], EXP, bias=uwu[:])
            if b == 0:
                for ho in hout_late:
                    add_dep_helper(ho.ins, exp_inst.ins, sync=True,
                                   reason="delay h passthrough into DMA lull")

            # re-transpose ET (4 tiles into one bank); batched 3D reduces
            et = ps.tile([P, TPB * P], f32r, tag="tp")
            for q in range(TPB):
                t = b * TPB + q
                nc.tensor.transpose(
                    et[:, ds(q * P, P)], ET[:, ds(t * P, P)], ident_r[:]
                )
            et3 = et[:].rearrange("p (q x) -> p q x", q=TPB)
            nc.vector.reduce_max(m_exp[:, ds(b * TPB, TPB)], et3, axis=AX)
            zsum = stage.tile([P, TPB], f32, tag="zs")
            nc.vector.reduce_sum(zsum[:], et3, axis=AX)
            nc.vector.reciprocal(z_rec[:, ds(b * TPB, TPB)], zsum[:])
            nc.scalar.copy(m_exp_r[:, ds(b * TPB, TPB)], m_exp[:, ds(b * TPB, TPB)])

            # q2c accumulation for this block's tiles (single PSUM group
            # spanning both blocks; other matmuls interleave freely)
            for q in range(TPB):
                t = b * TPB + q
                nc.tensor.matmul(
                    hap[:], m_exp_r[:, ds(t, 1)], h_r[:, ds(t * D, D)],
                    start=(b == 0 and q == 0), stop=(b == NB - 1 and q == TPB - 1),
                    skip_group_check=True,
                )
            if b == NB - 1:
                # q2c chain emitted ahead of the last c2q loop: bc becomes
                # ready while stg work still streams, shortening the tail
                mrow = consts.tile([P, 1], f32)
                nc.vector.reduce_sum(mrow[:], m_exp[:], axis=AX)
                zqp = ps.tile([1, 1], f32, tag="acc", bufs=1)
                nc.tensor.matmul(zqp[:], mrow[:], ones_col[:], start=True, stop=True)
                rzq = consts.tile([1, 1], f32)
                nc.vector.reciprocal(rzq[:], zqp[:])
                ha_sum = consts.tile([1, D], f32)
                nc.vector.tensor_copy(ha_sum[:], hap[:])
                ha_row = consts.tile([1, D], f32r)
                nc.scalar.mul(ha_row[:], ha_sum[:], rzq[:])
                bc = ps.tile([P, D], f32, tag="acc", bufs=1)
                nc.tensor.matmul(bc[:], ones_row_r[:], ha_row[:], start=True, stop=True)

            for q in range(TPB):
                t = b * TPB + q
                up = ps.tile([P, D], f32, tag="ua")
                nc.tensor.matmul(
                    up[:], ET[:, ds(t * P, P)], u_r[:], start=True, stop=True
                )
                stg = stage.tile([P, 2 * D], f32, tag="stg")
                nc.scalar.mul(stg[:, ds(0, D)], up[:], z_rec[:, ds(t, 1)])
                nc.vector.scalar_tensor_tensor(
                    stg[:, ds(D, D)], up[:], z_rec[:, ds(t, 1)], h_all[:, ds(t * D, D)],
                    op0=mybir.AluOpType.mult, op1=mybir.AluOpType.mult,
                )
                nc.sync.dma_start(out_d[ds(t * P, P), ds(D, 2 * D)], stg[:])
                if b == NB - 1:
                    # interleave o4 tiles after each stg tile
                    for tt in range(q * (NT // TPB), (q + 1) * (NT // TPB)):
                        o4 = stage.tile([P, D], f32, tag="o4")
                        nc.vector.tensor_mul(o4[:], h_all[:, ds(tt * D, D)], bc[:])
                        nc.sync.dma_start(out_d[ds(tt * P, P), ds(3 * D, D)], o4[:])

    nc.compile()
    return nc


def _get_nc():
    if "nc" not in _CACHE:
        _CACHE["nc"] = _build_program()
    return _CACHE["nc"]


def _ensure_axon_hooks_stub():
    # concourse imports antenv.axon_hooks when tracing is requested via env;
    # provide a no-op stub if the image lacks it so runs degrade gracefully.
    import sys
    import types

    try:
        import antenv.axon_hooks  # noqa: F401
    except ImportError:
        mod = types.ModuleType("antenv.axon_hooks")
        _hook = [None]
        mod.set_axon_ntff_profile_hook = lambda hook: _hook.__setitem__(0, hook)
        mod.get_axon_ntff_profile_hook = lambda: _hook[0]
        sys.modules["antenv.axon_hooks"] = mod


def kernel(h, u, alpha_w, alpha_b=None, **_unused):
    _ensure_axon_hooks_stub()
    from concourse.bass_utils import run_bass_kernel_spmd

    h = np.ascontiguousarray(np.asarray(h, dtype=np.float32)).reshape(N_B, JX, D)
    u = np.ascontiguousarray(np.asarray(u, dtype=np.float32)).reshape(N_B, JQ, D)
    alpha_w = np.ascontiguousarray(np.asarray(alpha_w, dtype=np.float32)).reshape(3 * D)

    nc = _get_nc()
    in_maps = [
        {"h": h[n], "u": u[n], "alpha_w": alpha_w} for n in range(N_B)
    ]
    res = run_bass_kernel_spmd(nc, in_maps, core_ids=list(range(N_B)))
    out = np.stack([res.results[n]["out"] for n in range(N_B)], axis=0)
    return out.reshape(N_B, M_B, JX, 4 * D)



# revision 4
# speedup vs baseline: 1.0879x; 1.0879x over previous
"""Trainium2 Bass kernel for BiDAF-style bidirectional attention.

Reference computation (per batch element n; M=1 folded away):
    s[i,j]  = h[i].w_h + u[j].w_u + (h[i]*u[j]).w_hu + b      [JX, JQ]
    a_u     = softmax_j(s);     u_a[i] = sum_j a_u[i,j] u[j]   (c2q)
    a_h     = softmax_i(max_j s);  h_a = sum_i a_h[i] h[i]     (q2c)
    out     = concat(h, u_a, h*u_a, h*h_a)                     [JX, 4D]

Sharding: data-parallel over batch N=8, one NeuronCore per batch element.
alpha_b drops out (both softmaxes are shift-invariant), accepted but unused.

Per-core I/O is 10.25 MiB (h 2 + u 0.25 + out 8), i.e. ~30 us at the 358 GB/s
per-core HBM roofline, so the schedule is built around keeping the store
stream continuous from ~4.5 us on:
  - 4 blocks of 2 i-tiles stream through scores->exp->c2q as h loads arrive
    (the c2q softmax over j is local to an i-row; only o4 = h*h_a needs the
    global max/softmax over all JX).
  - DMA count is minimized (each dma_start costs ~0.7 us of sequencer issue
    time): h as 4 x 512 KiB loads + u (sync), alpha_w broadcasts (gpsimd),
    o1 = h passthrough as 2 x 1 MiB bulk stores (gpsimd, SWDGE), per-block
    [o2|o3] slabs as 4 x 1 MiB stores and o4 as 2 x 1 MiB stores (sync).
  - f32 tiles are bitcast to f32r at matmul use sites (no cast copies).
  - scores are computed TRANSPOSED per block: sT[j,i] = sum_d uwT[d,j]hT[d,i]
    over 4 d-chunks, + h.w_h via a K=1 matmul (ones_col x hwh_row), u.w_u as
    the per-partition bias of the Exp eviction.  PE re-transposes ET tiles so
    DVE 3D-reduces give per-i max (q2c weight, exact) and 1/rowsum.
  - a short PE warmup burst lifts the HAM clock gate (1.2 -> 2.4 GHz) while
    the first h DMAs are in flight; per-block matmul pressure keeps it warm
    through the compute wave.
"""

import numpy as np

N_B, M_B, JX, JQ, D = 8, 1, 1024, 128, 512
P = 128
NT = JX // P    # 8 i-tiles
KC = D // P     # 4 d-chunks
TPB = 2         # tiles per block
NB = NT // TPB  # 4 blocks
IB = TPB * P    # 256 i per block

_CACHE = {}


def _build_program():
    from contextlib import ExitStack

    import concourse.bass as bass
    import concourse.tile as tile
    from concourse import bacc, mybir
    from concourse.masks import make_identity

    f32 = mybir.dt.float32
    f32r = mybir.dt.float32r
    EXP = mybir.ActivationFunctionType.Exp
    AX = mybir.AxisListType.X
    ds = bass.ds

    nc = bacc.Bacc("TRN2", target_bir_lowering=False, debug=False, num_devices=8)
    h_d = nc.dram_tensor("h", [JX, D], f32, kind="ExternalInput").ap()
    u_d = nc.dram_tensor("u", [JQ, D], f32, kind="ExternalInput").ap()
    aw_d = nc.dram_tensor("alpha_w", [3 * D], f32, kind="ExternalInput").ap()
    out_d = nc.dram_tensor("out", [JX, 4 * D], f32, kind="ExternalOutput").ap()

    with tile.TileContext(nc) as tc, ExitStack() as ctx:
        consts = ctx.enter_context(tc.tile_pool(name="consts", bufs=1))
        slab = ctx.enter_context(tc.tile_pool(name="slab", bufs=3))
        # PSUM (8 banks): tp=2, s/et=2, ua=2 (warmup + c2q), hap=1, misc=1
        ps = ctx.enter_context(tc.tile_pool(name="ps", bufs=2, space="PSUM"))

        # ---- input DMAs (sync = HWDGE; issue order is drain order) ----
        u_sb = consts.tile([JQ, D], f32)
        nc.sync.dma_start(u_sb[:], u_d[:])
        h_all = consts.tile([P, NT * D], f32)   # tile t: h[t*128+p, d]
        h3 = h_all[:].rearrange("p (t d) -> p t d", t=NT)
        for q in range(NT // 2):
            nc.sync.dma_start(
                h3[:, ds(2 * q, 2), :],
                h_d[ds(2 * q * P, 2 * P), :].rearrange("(t p) d -> p t d", p=P),
            )

        # alpha_w loads on gpsimd (SWDGE) to keep sync free for h
        wb = consts.tile([P, 3 * D], f32)  # [w_h | w_u | w_hu] bcast over parts
        nc.gpsimd.dma_start(
            wb[:], aw_d[:].rearrange("(o d) -> o d", o=1).to_broadcast((P, 3 * D))
        )
        w_cols = consts.tile([P, 12], f32)  # alpha_w partition-major: d = c*128+p
        nc.gpsimd.dma_start(w_cols[:], aw_d.rearrange("(c p) -> p c", p=P))
        wu_b = wb[:, ds(D, D)]
        whu_b = wb[:, ds(2 * D, D)]

        # ---- constants ----
        warm_f = consts.tile([P, D], f32)
        nc.vector.memset(warm_f[:], 0.25)
        warm_r = consts.tile([P, D], f32r)
        nc.vector.tensor_copy(warm_r[:], warm_f[:])
        ones_row = consts.tile([1, P], f32)
        nc.vector.memset(ones_row[:], 1.0)
        ones_row_r = consts.tile([1, P], f32r)
        nc.scalar.copy(ones_row_r[:], ones_row[:])
        ones_col = consts.tile([P, 1], f32)
        nc.vector.memset(ones_col[:], 1.0)
        ident = consts.tile([P, P], f32)
        make_identity(nc, ident[:])
        ident_r = consts.tile([P, P], f32r)
        nc.vector.tensor_copy(ident_r[:], ident[:])
        w_cols_r = consts.tile([P, 12], f32r)
        nc.vector.tensor_copy(w_cols_r[:], w_cols[:])

        # ---- PE warmup: open the HAM clock gate while h streams in ----
        for w in range(6):
            wp = ps.tile([P, D], f32, tag="ua")
            nc.tensor.matmul(wp[:], warm_r[:, ds(0, P)], warm_r[:],
                             start=True, stop=True)

        # ---- u prep ----
        u_r = consts.tile([JQ, D], f32r)
        nc.scalar.copy(u_r[:], u_sb[:])
        uw = consts.tile([JQ, D], f32)   # u * w_hu
        nc.vector.tensor_mul(uw[:], u_sb[:], whu_b)
        uwtmp = consts.tile([JQ, D], f32)
        uwu = consts.tile([JQ, 1], f32)  # per-j bias: sum_d u*w_u
        nc.vector.scalar_tensor_tensor(
            uwtmp[:], u_sb[:], 1.0, wu_b,
            op0=mybir.AluOpType.mult, op1=mybir.AluOpType.mult, accum_out=uwu[:],
        )
        # uwT[d, j] per chunk: 4 transposes into one bank, one batched evict
        uwT = consts.tile([P, KC * JQ], f32r)
        pt0 = ps.tile([P, KC * P], f32, tag="tp")
        for k in range(KC):
            nc.tensor.transpose(pt0[:, ds(k * P, P)], uw[:, ds(k * P, P)], ident[:])
        nc.scalar.copy(uwT[:], pt0[:])

        # ---- per-block streaming pipeline ----
        hT_all = consts.tile([P, KC * JX], f32r)  # chunk k: hT[k*128+p, i]
        hT3 = hT_all[:].rearrange("p (k x) -> p k x", k=KC)
        ET = consts.tile([JQ, JX], f32r)          # exp(sT + uwu + hwh)
        hwh_sb = consts.tile([1, JX], f32r)       # h . w_h (row over i)
        m_exp = consts.tile([P, NT], f32)         # per i-tile col: max_j ET
        z_rec = consts.tile([P, NT], f32)         # per i-tile col: 1/sum_j ET
        zsum = consts.tile([P, NT], f32)
        o4_sb = consts.tile([P, NT * D], f32)
        hap = ps.tile([1, D], f32, tag="hap", bufs=1)

        for b in range(NB):
            t0 = b * TPB
            blk = ds(b * IB, IB)
            # transposes of the block's h tiles: one PSUM bank per tile
            for q in range(TPB):
                t = t0 + q
                pt = ps.tile([P, KC * P], f32, tag="tp")
                for k in range(KC):
                    nc.tensor.transpose(
                        pt[:, ds(k * P, P)], h_all[:, ds(t * D + k * P, P)], ident[:]
                    )
                ev = nc.scalar.copy if q == 0 else nc.vector.tensor_copy
                ev(hT3[:, :, ds(t * P, P)], pt[:].rearrange("p (k x) -> p k x", k=KC))

            # hwh row for the block: [1, IB] accumulated over d-chunks
            hp = ps.tile([1, IB], f32, tag="misc", bufs=1)
            for k in range(KC):
                nc.tensor.matmul(
                    hp[:], w_cols_r[:, ds(k, 1)], hT_all[:, ds(k * JX + b * IB, IB)],
                    start=(k == 0), stop=(k == KC - 1),
                )
            nc.scalar.copy(hwh_sb[:, blk], hp[:])

            # transposed scores: sT[j, i] += uwT_k^T @ hT_k + ones x hwh
            sp = ps.tile([JQ, IB], f32, tag="set")
            for k in range(KC):
                nc.tensor.matmul(
                    sp[:], uwT[:, ds(k * JQ, JQ)],
                    hT_all[:, ds(k * JX + b * IB, IB)],
                    start=(k == 0), stop=False,
                )
            nc.tensor.matmul(
                sp[:], ones_row_r[:], hwh_sb[:, blk], start=False, stop=True
            )
            # ET = exp(sT + uwu[j]) with uwu as the per-partition ACT bias
            nc.scalar.activation(ET[:, blk], sp[:], EXP, bias=uwu[:])

            # re-transpose ET tiles; batched 3D reduces -> max, 1/sum per i
            et = ps.tile([P, IB], f32r, tag="set")
            for q in range(TPB):
                t = t0 + q
                nc.tensor.transpose(et[:, ds(q * P, P)], ET[:, ds(t * P, P)], ident_r[:])
            et3 = et[:].rearrange("p (q x) -> p q x", q=TPB)
            nc.vector.reduce_max(m_exp[:, ds(t0, TPB)], et3, axis=AX)
            nc.vector.reduce_sum(zsum[:, ds(t0, TPB)], et3, axis=AX)
            nc.vector.reciprocal(z_rec[:, ds(t0, TPB)], zsum[:, ds(t0, TPB)])

            # q2c accumulation (single PSUM group spanning all blocks)
            for q in range(TPB):
                t = t0 + q
                nc.tensor.matmul(
                    hap[:], m_exp[:, ds(t, 1)], h_all[:, ds(t * D, D)],
                    start=(t == 0), stop=(t == NT - 1),
                    skip_group_check=True,
                )

            # c2q per tile: u_a = (ET_t^T @ u) * rz ; o3 = u_a * h
            stg = slab.tile([P, TPB * 2 * D], f32, tag="stg")
            for q in range(TPB):
                t = t0 + q
                up = ps.tile([P, D], f32, tag="ua")
                nc.tensor.matmul(up[:], ET[:, ds(t * P, P)], u_r[:],
                                 start=True, stop=True)
                o2 = stg[:, ds(q * 2 * D, D)]
                nc.scalar.mul(o2, up[:], z_rec[:, ds(t, 1)])
                nc.gpsimd.tensor_mul(
                    stg[:, ds(q * 2 * D + D, D)], o2, h_all[:, ds(t * D, D)]
                )
            nc.sync.dma_start(
                out_d[ds(b * IB, IB), ds(D, 2 * D)].rearrange(
                    "(t p) d -> p t d", p=P
                ),
                stg[:].rearrange("p (t d) -> p t d", t=TPB),
            )

            # o1 = h passthrough: 2 bulk stores once each half of h is here
            if b == 1 or b == 3:
                half = ds((b - 1) * 2 * D, 4 * D)
                nc.gpsimd.dma_start(
                    out_d[ds((b - 1) * 2 * P, 4 * P), ds(0, D)].rearrange(
                        "(t p) d -> p t d", p=P
                    ),
                    h_all[:, half].rearrange("p (t d) -> p t d", t=4),
                )

        # ---- q2c tail: h_a broadcast + o4 ----
        mrow = consts.tile([P, 1], f32)
        nc.vector.reduce_sum(mrow[:], m_exp[:], axis=AX)
        zqp = ps.tile([1, 1], f32, tag="misc", bufs=1)
        nc.tensor.matmul(zqp[:], mrow[:], ones_col[:], start=True, stop=True)
        rzq = consts.tile([1, 1], f32)
        nc.vector.reciprocal(rzq[:], zqp[:])
        ha_row = consts.tile([1, D], f32r)
        nc.scalar.mul(ha_row[:], hap[:], rzq[:])
        bcp = ps.tile([P, D], f32, tag="misc", bufs=1)
        nc.tensor.matmul(bcp[:], ones_row_r[:], ha_row[:], start=True, stop=True)
        bc = consts.tile([P, D], f32)
        nc.scalar.copy(bc[:], bcp[:])

        for t in range(NT):
            eng = (nc.vector, nc.gpsimd)[t % 2]
            eng.tensor_mul(o4_sb[:, ds(t * D, D)], h_all[:, ds(t * D, D)], bc[:])
            if t == 3 or t == 7:
                half = ds((t - 3) * D, 4 * D)
                nc.sync.dma_start(
                    out_d[ds((t - 3) * P, 4 * P), ds(3 * D, D)].rearrange(
                        "(t p) d -> p t d", p=P
                    ),
                    o4_sb[:, half].rearrange("p (t d) -> p t d", t=4),
                )

    nc.compile()
    return nc


def _get_nc():
    if "nc" not in _CACHE:
        _CACHE["nc"] = _build_program()
    return _CACHE["nc"]


def _ensure_axon_hooks_stub():
    # concourse imports antenv.axon_hooks when tracing is requested via env;
    # provide a no-op stub if the image lacks it so runs degrade gracefully.
    import sys
    import types

    try:
        import antenv.axon_hooks  # noqa: F401
    except ImportError:
        mod = types.ModuleType("antenv.axon_hooks")
        _hook = [None]
        mod.set_axon_ntff_profile_hook = lambda hook: _hook.__setitem__(0, hook)
        mod.get_axon_ntff_profile_hook = lambda: _hook[0]
        sys.modules["antenv.axon_hooks"] = mod


def kernel(h, u, alpha_w, alpha_b=None, **_unused):
    _ensure_axon_hooks_stub()
    from concourse.bass_utils import run_bass_kernel_spmd

    h = np.ascontiguousarray(np.asarray(h, dtype=np.float32)).reshape(N_B, JX, D)
    u = np.ascontiguousarray(np.asarray(u, dtype=np.float32)).reshape(N_B, JQ, D)
    alpha_w = np.ascontiguousarray(np.asarray(alpha_w, dtype=np.float32)).reshape(3 * D)

    nc = _get_nc()
    in_maps = [
        {"h": h[n], "u": u[n], "alpha_w": alpha_w} for n in range(N_B)
    ]
    res = run_bass_kernel_spmd(nc, in_maps, core_ids=list(range(N_B)))
    out = np.stack([res.results[n]["out"] for n in range(N_B)], axis=0)
    return out.reshape(N_B, M_B, JX, 4 * D)


# revision 7
# speedup vs baseline: 1.1603x; 1.0666x over previous
"""Trainium2 Bass kernel for BiDAF-style bidirectional attention.

Reference computation (per batch element n; M=1 folded away):
    s[i,j]  = h[i].w_h + u[j].w_u + (h[i]*u[j]).w_hu + b      [JX, JQ]
    a_u     = softmax_j(s);     u_a[i] = sum_j a_u[i,j] u[j]   (c2q)
    a_h     = softmax_i(max_j s);  h_a = sum_i a_h[i] h[i]     (q2c)
    out     = concat(h, u_a, h*u_a, h*h_a)                     [JX, 4D]

Sharding: data-parallel over batch N=8, one NeuronCore per batch element.
alpha_b drops out (both softmaxes are shift-invariant), accepted but unused.

Per-core I/O is 10.25 MiB (h 2 + u 0.25 + out 8), i.e. ~30 us at the 358 GB/s
per-core HBM roofline, so the schedule is built around keeping the store
stream continuous from ~4.5 us on:
  - 4 blocks of 2 i-tiles stream through scores->exp->c2q as h loads arrive
    (the c2q softmax over j is local to an i-row; only o4 = h*h_a needs the
    global max/softmax over all JX).
  - DMA count is minimized (each dma_start costs ~0.7 us of sequencer issue
    time): h as 4 x 512 KiB loads + u (sync), alpha_w broadcasts (gpsimd),
    o1 = h passthrough as 2 x 1 MiB bulk stores (gpsimd, SWDGE), per-block
    [o2|o3] slabs as 4 x 1 MiB stores and o4 as 2 x 1 MiB stores (sync).
  - f32 tiles are bitcast to f32r at matmul use sites (no cast copies).
  - scores are computed TRANSPOSED per block: sT[j,i] = sum_d uwT[d,j]hT[d,i]
    over 4 d-chunks, + h.w_h via a K=1 matmul (ones_col x hwh_row), u.w_u as
    the per-partition bias of the Exp eviction.  PE re-transposes ET tiles so
    DVE 3D-reduces give per-i max (q2c weight, exact) and 1/rowsum.
  - a short PE warmup burst lifts the HAM clock gate (1.2 -> 2.4 GHz) while
    the first h DMAs are in flight; per-block matmul pressure keeps it warm
    through the compute wave.
"""

import numpy as np

N_B, M_B, JX, JQ, D = 8, 1, 1024, 128, 512
P = 128
NT = JX // P    # 8 i-tiles
KC = D // P     # 4 d-chunks
TPB = 2         # tiles per block
NB = NT // TPB  # 4 blocks
IB = TPB * P    # 256 i per block

_CACHE = {}


def _build_program():
    from contextlib import ExitStack

    import concourse.bass as bass
    import concourse.tile as tile
    from concourse import bacc, mybir
    from concourse.masks import make_identity

    f32 = mybir.dt.float32
    f32r = mybir.dt.float32r
    EXP = mybir.ActivationFunctionType.Exp
    AX = mybir.AxisListType.X
    ds = bass.ds

    nc = bacc.Bacc("TRN2", target_bir_lowering=False, debug=False, num_devices=8)
    h_d = nc.dram_tensor("h", [JX, D], f32, kind="ExternalInput").ap()
    u_d = nc.dram_tensor("u", [JQ, D], f32, kind="ExternalInput").ap()
    aw_d = nc.dram_tensor("alpha_w", [3 * D], f32, kind="ExternalInput").ap()
    out_d = nc.dram_tensor("out", [JX, 4 * D], f32, kind="ExternalOutput").ap()

    with tile.TileContext(nc) as tc, ExitStack() as ctx:
        consts = ctx.enter_context(tc.tile_pool(name="consts", bufs=1))
        slab = ctx.enter_context(tc.tile_pool(name="slab", bufs=3))
        # PSUM (8 banks): tp=3, s/et=2, ua=2 (warmup + c2q + bc), hap=1
        ps = ctx.enter_context(tc.tile_pool(name="ps", bufs=2, space="PSUM"))

        # ---- input DMAs (sync = HWDGE; issue order is drain order) ----
        u_sb = consts.tile([JQ, D], f32)
        nc.sync.dma_start(u_sb[:], u_d[:])
        h_all = consts.tile([P, NT * D], f32)   # tile t: h[t*128+p, d]
        h3 = h_all[:].rearrange("p (t d) -> p t d", t=NT)
        for q in range(NT // 2):
            nc.sync.dma_start(
                h3[:, ds(2 * q, 2), :],
                h_d[ds(2 * q * P, 2 * P), :].rearrange("(t p) d -> p t d", p=P),
            )

        # alpha_w loads on scalar (also HWDGE) to keep sync free for h
        w_cols = consts.tile([P, 12], f32)  # alpha_w partition-major: d = c*128+p
        nc.scalar.dma_start(w_cols[:], aw_d.rearrange("(c p) -> p c", p=P))
        wu_bc = consts.tile([P, D], f32)   # w_u broadcast over partitions
        nc.scalar.dma_start(
            wu_bc[:], aw_d[ds(D, D)].rearrange("(o d) -> o d", o=1).to_broadcast((P, D))
        )

        # ---- constants ----
        warm_f = consts.tile([P, D], f32)
        nc.vector.memset(warm_f[:], 0.25)
        warm_r = consts.tile([P, D], f32r)
        nc.vector.tensor_copy(warm_r[:], warm_f[:])
        ones_row = consts.tile([1, P], f32)
        nc.vector.memset(ones_row[:], 1.0)
        ones_row_r = consts.tile([1, P], f32r)
        nc.scalar.copy(ones_row_r[:], ones_row[:])
        ones_col = consts.tile([P, 1], f32)
        nc.vector.memset(ones_col[:], 1.0)
        ident = consts.tile([P, P], f32)
        make_identity(nc, ident[:])
        ident_r = consts.tile([P, P], f32r)
        nc.vector.tensor_copy(ident_r[:], ident[:])

        # ---- PE warmup: open the HAM clock gate while h streams in ----
        for w in range(4):
            wp = ps.tile([P, D], f32, tag="ua")
            nc.tensor.matmul(wp[:], warm_r[:, ds(0, P)], warm_r[:],
                             start=True, stop=True)

        # ---- u prep ----
        u_r = consts.tile([JQ, D], f32r)
        nc.scalar.copy(u_r[:], u_sb[:])
        uwtmp = consts.tile([JQ, D], f32)
        uwu = consts.tile([JQ, 1], f32)  # per-j bias: sum_d u*w_u
        nc.vector.scalar_tensor_tensor(
            uwtmp[:], u_sb[:], 1.0, wu_bc[:],
            op0=mybir.AluOpType.mult, op1=mybir.AluOpType.mult, accum_out=uwu[:],
        )
        # uwT'[d, j] = u[j,d]*w_hu[d] + w_h[d]: transpose u, then fold the
        # scale/bias into the PSUM eviction.  The w_h term makes the score
        # matmul emit s + h.w_h directly (hwh[i] = sum_d w_h[d] hT[d,i]).
        uwT = consts.tile([P, KC * JQ], f32r)
        pt0 = ps.tile([P, KC * P], f32, tag="tp", bufs=3)
        for k in range(KC):
            nc.tensor.transpose(pt0[:, ds(k * P, P)], u_sb[:, ds(k * P, P)], ident[:])
        IDENT_F = mybir.ActivationFunctionType.Identity
        for k in range(KC):
            nc.scalar.activation(
                uwT[:, ds(k * JQ, JQ)], pt0[:, ds(k * P, P)], IDENT_F,
                bias=w_cols[:, ds(k, 1)], scale=w_cols[:, ds(8 + k, 1)],
            )

        # ---- per-block streaming pipeline ----
        hT_all = consts.tile([P, KC * JX], f32r)  # chunk k: hT[k*128+p, i]
        hT3 = hT_all[:].rearrange("p (k x) -> p k x", k=KC)
        ET = consts.tile([JQ, JX], f32r)          # exp(sT + uwu + hwh)
        m_exp = consts.tile([P, NT], f32)         # per i-tile col: max_j ET
        z_rec = consts.tile([P, NT], f32)         # per i-tile col: 1/sum_j ET
        zsum = consts.tile([P, NT], f32)
        o4_sb = consts.tile([P, NT * D], f32)
        hap = ps.tile([P, D], f32, tag="hap", bufs=1)

        for b in range(NB):
            t0 = b * TPB
            blk = ds(b * IB, IB)
            # transposes of the block's h tiles: one PSUM bank per tile
            for q in range(TPB):
                t = t0 + q
                pt = ps.tile([P, KC * P], f32, tag="tp", bufs=3)
                for k in range(KC):
                    nc.tensor.transpose(
                        pt[:, ds(k * P, P)], h_all[:, ds(t * D + k * P, P)], ident[:]
                    )
                ev = nc.scalar.copy if q == 0 else nc.vector.tensor_copy
                ev(hT3[:, :, ds(t * P, P)], pt[:].rearrange("p (k x) -> p k x", k=KC))

            # transposed scores: sT[j, i] = sum_k uwT'_k^T @ hT_k  (incl. h.w_h)
            sp = ps.tile([JQ, IB], f32, tag="set")
            for k in range(KC):
                nc.tensor.matmul(
                    sp[:], uwT[:, ds(k * JQ, JQ)],
                    hT_all[:, ds(k * JX + b * IB, IB)],
                    start=(k == 0), stop=(k == KC - 1),
                )
            # ET = exp(sT + uwu[j]) with uwu as the per-partition ACT bias
            nc.scalar.activation(ET[:, blk], sp[:], EXP, bias=uwu[:])

            # re-transpose ET tiles; batched 3D reduces -> max, 1/sum per i
            et = ps.tile([P, IB], f32r, tag="set")
            for q in range(TPB):
                t = t0 + q
                nc.tensor.transpose(et[:, ds(q * P, P)], ET[:, ds(t * P, P)], ident_r[:])
            et3 = et[:].rearrange("p (q x) -> p q x", q=TPB)
            nc.vector.reduce_max(m_exp[:, ds(t0, TPB)], et3, axis=AX)
            nc.vector.reduce_sum(zsum[:, ds(t0, TPB)], et3, axis=AX)
            nc.vector.reciprocal(z_rec[:, ds(t0, TPB)], zsum[:, ds(t0, TPB)])

            # q2c accumulation: fp32 matmuls col-tiled to groups t%2 so each
            # block's pair runs concurrently; 2 partial rows summed at the end
            for q in range(TPB):
                t = t0 + q
                g = t % 2
                nc.tensor.matmul(
                    hap[ds(32 * g, 1), :], m_exp[:, ds(t, 1)], h_all[:, ds(t * D, D)],
                    start=(t < 2), stop=(t >= NT - 2),
                    skip_group_check=True, tile_position=(0, 32 * g),
                )

            # c2q per tile: u_a = (ET_t^T @ u) * rz ; o3 = u_a * h
            stg = slab.tile([P, TPB * 2 * D], f32, tag="stg")
            for q in range(TPB):
                t = t0 + q
                up = ps.tile([P, D], f32, tag="ua")
                nc.tensor.matmul(up[:], ET[:, ds(t * P, P)], u_r[:],
                                 start=True, stop=True)
                nc.scalar.mul(stg[:, ds(q * 2 * D, D)], up[:], z_rec[:, ds(t, 1)])
                nc.vector.scalar_tensor_tensor(
                    stg[:, ds(q * 2 * D + D, D)], up[:], z_rec[:, ds(t, 1)],
                    h_all[:, ds(t * D, D)],
                    op0=mybir.AluOpType.mult, op1=mybir.AluOpType.mult,
                )
            nc.sync.dma_start(
                out_d[ds(b * IB, IB), ds(D, 2 * D)].rearrange(
                    "(t p) d -> p t d", p=P
                ),
                stg[:].rearrange("p (t d) -> p t d", t=TPB),
            )

            # o1 = h passthrough: 2 bulk stores once each half of h is here
            if b == 1 or b == 3:
                half = ds((b - 1) * 2 * D, 4 * D)
                nc.gpsimd.dma_start(
                    out_d[ds((b - 1) * 2 * P, 4 * P), ds(0, D)].rearrange(
                        "(t p) d -> p t d", p=P
                    ),
                    h_all[:, half].rearrange("p (t d) -> p t d", t=4),
                )

        # ---- q2c tail: h_a broadcast + o4 ----
        mrow = consts.tile([P, 1], f32)
        nc.vector.reduce_sum(mrow[:], m_exp[:], axis=AX)
        zqp = ps.tile([1, 1], f32, tag="set")
        nc.tensor.matmul(zqp[:], mrow[:], ones_col[:], start=True, stop=True)
        rzq = consts.tile([1, 1], f32)
        nc.vector.reciprocal(rzq[:], zqp[:])
        ha_row0 = consts.tile([1, D], f32r)
        ha_row1 = consts.tile([1, D], f32r)
        nc.scalar.mul(ha_row0[:], hap[ds(0, 1), :], rzq[:])
        nc.vector.tensor_scalar_mul(ha_row1[:], hap[ds(32, 1), :], rzq[:])
        bcp = ps.tile([P, D], f32, tag="ua")
        nc.tensor.matmul(bcp[:], ones_row_r[:], ha_row0[:], start=True, stop=False)
        nc.tensor.matmul(bcp[:], ones_row_r[:], ha_row1[:], start=False, stop=True)
        bc = consts.tile([P, D], f32)
        nc.scalar.copy(bc[:], bcp[:])

        for t in range(NT):
            nc.vector.tensor_mul(o4_sb[:, ds(t * D, D)], h_all[:, ds(t * D, D)], bc[:])
            if t == 3 or t == 7:
                half = ds((t - 3) * D, 4 * D)
                nc.sync.dma_start(
                    out_d[ds((t - 3) * P, 4 * P), ds(3 * D, D)].rearrange(
                        "(t p) d -> p t d", p=P
                    ),
                    o4_sb[:, half].rearrange("p (t d) -> p t d", t=4),
                )

    nc.compile()
    return nc


def _get_nc():
    if "nc" not in _CACHE:
        _CACHE["nc"] = _build_program()
    return _CACHE["nc"]


def _ensure_axon_hooks_stub():
    # concourse imports antenv.axon_hooks when tracing is requested via env;
    # provide a no-op stub if the image lacks it so runs degrade gracefully.
    import sys
    import types

    try:
        import antenv.axon_hooks  # noqa: F401
    except ImportError:
        mod = types.ModuleType("antenv.axon_hooks")
        _hook = [None]
        mod.set_axon_ntff_profile_hook = lambda hook: _hook.__setitem__(0, hook)
        mod.get_axon_ntff_profile_hook = lambda: _hook[0]
        sys.modules["antenv.axon_hooks"] = mod


def kernel(h, u, alpha_w, alpha_b=None, **_unused):
    _ensure_axon_hooks_stub()
    from concourse.bass_utils import run_bass_kernel_spmd

    h = np.ascontiguousarray(np.asarray(h, dtype=np.float32)).reshape(N_B, JX, D)
    u = np.ascontiguousarray(np.asarray(u, dtype=np.float32)).reshape(N_B, JQ, D)
    alpha_w = np.ascontiguousarray(np.asarray(alpha_w, dtype=np.float32)).reshape(3 * D)

    nc = _get_nc()
    in_maps = [
        {"h": h[n], "u": u[n], "alpha_w": alpha_w} for n in range(N_B)
    ]
    res = run_bass_kernel_spmd(nc, in_maps, core_ids=list(range(N_B)))
    out = np.stack([res.results[n]["out"] for n in range(N_B)], axis=0)
    return out.reshape(N_B, M_B, JX, 4 * D)
